# revision 36
# baseline (speedup 1.0000x reference)
"""Trainium2 Bass kernel for nn_EnhancedUVMDModel.

Math: the UVMD Gauss-Seidel scan is linear in X = rfft(x) with real,
per-frequency coefficients, so the whole scan collapses to 4 real transfer
functions H_k(f) computed by a tiny O(K*F) host recurrence.  In the time
domain each mode is a banded circular convolution of x with h_k = irfft(H_k),
evaluated as a banded block-Toeplitz matmul (128x128 blocks, half-width nd).

Per core (8 of 64 batch samples, pure data parallel):
  phase M: modes for all (k,b) via block-Toeplitz matmuls, evicted to fp8.
  conv stack: 16 groups of (k, 2 samples).  PSUM col-packing puts both
  samples of a group in one bank (pair A at partition 0, pair B at 64),
  halving eviction instruction count.  rhs for conv1/2/3 are im2col tiles
  built by SBUF->SBUF DMA in fp8 (weights stay bf16: mixed-dtype matmuls;
  fp8 weights would blow the error budget, fp8 activations cost ~3e-3).
  conv2 = contract 128 + 32 (im2col4 + offset trick), conv3 = 128 + 64
  (im2col2 + offset trick).  Evictions fuse bias+ReLU (BN folded) and are
  split DVE/ACT; conv3 eviction accumulates the time-pool via ACT accum_out.

Clocking: the PE HAM gate gives 2.4 GHz only while the PE never idles
>~3.4us; there is NO re-warm on this silicon path once throttled (observed:
750us of dense matmuls never re-warmed).  So the kernel keeps the PE stream
gap-free: host-packed dense input DMAs, filler matmuls across the
phaseM->conv pipeline fill, deep software pipelining, and im2col source
rows laid out c-major so DMA reads spread across SBUF AXI ports
(port = fixed group of 8 partitions; b-major concentrated 12 rows on ~3
ports and starved the pipeline).

All matmul operands sit at SBUF base partition 0 (tile_position row packing
wedges the device on this HW path; col packing `out[64:128]` is fine and
HW-validated).
"""
import numpy as np
import ml_dtypes

import concourse.bass as bass
import concourse.mybir as mybir
import concourse.tile as tile
from concourse import bacc

BF16 = ml_dtypes.bfloat16
E4M3 = ml_dtypes.float8_e4m3

NCORES = 8
B, T, C, K, L = 64, 4096, 12, 4, 8
BL = B // NCORES          # 8 samples per core
BC = BL * C               # 96 rows, b-major: row = 12b + c
F = T // 2 + 1            # 2049 rfft bins
NT = T // 512             # 8 time tiles
NBLK = T // 128           # 32 conv blocks
R6 = T + 6                # modesall row section (3+3 halo)
RH1 = T + 8               # h1dual cols (2 left, 6 right halo)
RH2 = T + 4               # h2dual cols (1 left, 3 right halo)
NG = 16                   # conv groups: (k, bpair)
BN_EPS = 1e-5

_NC_CACHE = {}


def _ap_with(base, dims, extra_offset=0):
    return bass.AP(base.tensor, base.offset + extra_offset, dims,
                   base.const_val, base.runtime_checks, base.dep_tracking_offset)


def _compute_H(alpha, tau, omega):
    """Real transfer functions H_k(f): u_k_final = H_k * X.  float64."""
    freqs = np.linspace(0.0, 0.5, F)
    a = np.zeros((K, F))
    bl = np.zeros(F)
    total = np.zeros(F)
    alpha = np.asarray(alpha, np.float64)
    tau = np.asarray(tau, np.float64)
    omega = np.asarray(omega, np.float64)
    for l in range(L):
        for k in range(K):
            resid = 1.0 - (total - a[k]) + bl / 2.0
            denom = 1.0 + alpha[l, k] * (freqs - omega[k]) ** 2
            new_a = resid / denom
            total = total - a[k] + new_a
            a[k] = new_a
        bl = bl + tau[l] * (1.0 - total)
    return a                                                      # (K, F)


def build_nc(nd=1):
    if nd in _NC_CACHE:
        return _NC_CACHE[nd]
    fp32 = mybir.dt.float32
    bf16 = mybir.dt.bfloat16
    f8e4 = mybir.dt.float8e4
    nc = bacc.Bacc()
    NDB = 2 * nd + 1
    KR6 = K * R6

    xh = nc.dram_tensor("xh", [128, 32 * BC], bf16, kind="ExternalInput")
    HBall = nc.dram_tensor("HBall", [128, NDB * 512], bf16, kind="ExternalInput")
    W1call = nc.dram_tensor("W1call", [84, K * 32], bf16, kind="ExternalInput")
    W2aall = nc.dram_tensor("W2aall", [128, K * 64], bf16, kind="ExternalInput")
    W2ball = nc.dram_tensor("W2ball", [32, K * 64], bf16, kind="ExternalInput")
    W3aall = nc.dram_tensor("W3aall", [128, K * 64], bf16, kind="ExternalInput")
    W3ball = nc.dram_tensor("W3ball", [64, K * 64], bf16, kind="ExternalInput")
    SB1d = nc.dram_tensor("SB1d", [64, K], fp32, kind="ExternalInput")
    SB2d = nc.dram_tensor("SB2d", [128, K], fp32, kind="ExternalInput")
    SB3d = nc.dram_tensor("SB3d", [128, K], fp32, kind="ExternalInput")
    Wc1m = nc.dram_tensor("Wc1m", [64, 512], fp32, kind="ExternalInput")
    bc1m = nc.dram_tensor("bc1m", [128, 1], fp32, kind="ExternalInput")
    Wc2m = nc.dram_tensor("Wc2m", [128, 10], fp32, kind="ExternalInput")
    bc2m = nc.dram_tensor("bc2m", [10, 1], fp32, kind="ExternalInput")
    out = nc.dram_tensor("out", [10, BL], fp32, kind="ExternalOutput")

    with tile.TileContext(nc) as tc:
        with (
            tc.tile_pool(name="persist", bufs=1) as pp,
            tc.tile_pool(name="wpool", bufs=1) as wp,
        ):
            xsb = pp.tile([128, 32 * BC], bf16, tag="xsb", name="xsb")
            modesall = pp.tile([BC, KR6], f8e4, tag="modesall", name="modesall")
            feat2 = pp.tile([128, NG], fp32, tag="feat2", name="feat2")
            featk = [pp.tile([64, BL], fp32, tag=f"featk{k}", name=f"featk{k}")
                     for k in range(K)]

            hball = wp.tile([128, NDB * 512], bf16, tag="hball", name="hball")
            w1c = wp.tile([84, K * 32], bf16, tag="w1c", name="w1c")
            w2a = wp.tile([128, K * 64], bf16, tag="w2a", name="w2a")
            w2b = wp.tile([32, K * 64], bf16, tag="w2b", name="w2b")
            w3a = wp.tile([128, K * 64], bf16, tag="w3a", name="w3a")
            w3b = wp.tile([64, K * 64], bf16, tag="w3b", name="w3b")
            sb1d = wp.tile([64, K], fp32, tag="sb1d", name="sb1d")
            sb2d = wp.tile([128, K], fp32, tag="sb2d", name="sb2d")
            sb3d = wp.tile([128, K], fp32, tag="sb3d", name="sb3d")
            wc1sb = wp.tile([64, 512], fp32, tag="wc1", name="wc1sb")
            bc1sb = wp.tile([128, 1], fp32, tag="bc1", name="bc1sb")
            wc2sb = wp.tile([128, 10], fp32, tag="wc2", name="wc2sb")
            bc2sb = wp.tile([10, 1], fp32, tag="bc2", name="bc2sb")

            # hball + the first-needed xsb blocks (phase M starts jj=-1 ->
            # j=31, then j=0,1,..) load first so phase M starts ~8us earlier
            nc.sync.dma_start(hball[:], HBall[:])
            nc.sync.dma_start(xsb[:, 31 * BC:32 * BC], xh[:, 31 * BC:32 * BC])
            nc.sync.dma_start(xsb[:, 0:8 * BC], xh[:, 0:8 * BC])
            nc.sync.dma_start(xsb[:, 8 * BC:31 * BC], xh[:, 8 * BC:31 * BC])
            nc.sync.dma_start(w1c[:], W1call[:])
            nc.sync.dma_start(w2a[:], W2aall[:])
            nc.sync.dma_start(w2b[:], W2ball[:])
            nc.sync.dma_start(w3a[:], W3aall[:])
            nc.sync.dma_start(w3b[:], W3ball[:])
            nc.sync.dma_start(sb1d[:], SB1d[:])
            nc.sync.dma_start(sb2d[:], SB2d[:])
            nc.sync.dma_start(sb3d[:], SB3d[:])
            nc.sync.dma_start(wc1sb[:], Wc1m[:])
            nc.sync.dma_start(bc1sb[:], bc1m[:])
            nc.sync.dma_start(wc2sb[:], Wc2m[:])
            nc.sync.dma_start(bc2sb[:], bc2m[:])

            # zero mode halos (3 cols each side of each k section)
            for k in range(K):
                nc.vector.memset(modesall[:, k * R6:k * R6 + 3], 0.0)
                nc.vector.memset(modesall[:, k * R6 + T + 3:(k + 1) * R6], 0.0)

            # sync bridges: touch constant tiles on DVE/ACT once so that
            # 1-wait-slot instructions only ever wait on one semaphore later.
            scrv = wp.tile([128, 8], fp32, tag="scrv", name="scrv")
            scrs = wp.tile([128, 8], fp32, tag="scrs", name="scrs")
            nc.vector.tensor_copy(scrv[0:64, 0:1], sb1d[:, 0:1])
            nc.vector.tensor_copy(scrv[:, 1:2], sb2d[:, 0:1])
            nc.vector.tensor_copy(scrv[:, 2:3], sb3d[:, 0:1])
            nc.scalar.copy(scrs[0:64, 0:1], sb1d[:, 0:1])
            nc.scalar.copy(scrs[:, 1:2], sb2d[:, 0:1])
            nc.scalar.copy(scrs[:, 2:3], sb3d[:, 0:1])
            nc.scalar.copy(scrs[:, 3:4], bc1sb[:])
            nc.scalar.copy(scrs[0:10, 4:5], bc2sb[:])

            # ---- Phase M: modes via banded block-Toeplitz circular conv ----
            with (
                tc.tile_pool(name="mpsum", bufs=6, space=bass.MemorySpace.PSUM) as mps,
            ):
                psm = {}
                ev_i = 0
                for jj in range(-nd, NBLK + nd):
                    j = jj % NBLK
                    for d in range(-nd, nd + 1):
                        i = jj - d
                        if not (0 <= i < NBLK):
                            continue
                        if i not in psm:
                            psm[i] = mps.tile([BC, 512], mybir.dt.float32,
                                              tag="mps", name=f"mps_{i}")
                        nc.tensor.matmul(
                            psm[i][:], xsb[:, BC * j:BC * (j + 1)],
                            hball[:, 512 * (d + nd):512 * (d + nd + 1)],
                            start=(d == -nd), stop=(d == nd))
                        if d == nd:
                            pt = psm.pop(i)
                            dst = _ap_with(modesall[:],
                                           [[KR6, BC], [R6, K], [1, 128]],
                                           extra_offset=3 + 128 * i)
                            if ev_i % 2 == 0:
                                nc.vector.tensor_copy(dst, pt[:])
                            else:
                                nc.scalar.copy(dst, pt[:])
                            ev_i += 1

            # ---- Conv stack: 16 groups of (k, 2 samples) ----
            with (
                tc.tile_pool(name="warmps", bufs=1, space=bass.MemorySpace.PSUM) as wps,
                tc.tile_pool(name="pc1p", bufs=3, space=bass.MemorySpace.PSUM) as pc1p,
                tc.tile_pool(name="pc2p", bufs=2, space=bass.MemorySpace.PSUM) as pc2p,
                tc.tile_pool(name="pc3p", bufs=2, space=bass.MemorySpace.PSUM) as pc3p,
                tc.tile_pool(name="convp", bufs=2) as cp,
            ):
                wpt = wps.tile([BC, 512], mybir.dt.float32, tag="warm", name="warmps")

                def filler(n):
                    # dependency-free warm matmuls (read xsb + hball only)
                    for wi in range(n):
                        nc.tensor.matmul(wpt[:],
                                         xsb[:, BC * (wi % NBLK):BC * (wi % NBLK) + BC],
                                         hball[:, 0:512],
                                         start=True, stop=True)

                rhs1_t, h1_t, rhs2_t, h2_t, rhs3_t, acc_t = {}, {}, {}, {}, {}, {}

                def gkb(g):
                    return g // 4, g % 4              # k, bpair

                def load1(g, eng=None):
                    k, bp = gkb(g)
                    eng = eng or nc.gpsimd
                    tiles = []
                    for hx in range(2):
                        b = 2 * bp + hx
                        r = cp.tile([84, T], f8e4, tag=f"rhs1{hx}", name=f"rhs1_{g}_{hx}", bufs=3)
                        eng.dma_start(
                            out=r[:],
                            in_=_ap_with(modesall[:], [[KR6, 12], [1, 7], [1, T]],
                                         extra_offset=12 * b * KR6 + k * R6))
                        tiles.append(r)
                    rhs1_t[g] = tiles

                def load2(g):
                    h1 = h1_t.pop(g)
                    tiles = []
                    for hx in range(2):
                        r = cp.tile([128, T + 4], f8e4, tag=f"rhs2{hx}",
                                    name=f"rhs2_{g}_{hx}", bufs=3)
                        for j in range(4):
                            nc.sync.dma_start(
                                out=r[32 * j:32 * (j + 1), :],
                                in_=_ap_with(h1[:], [[RH1, 32], [1, T + 4]],
                                             extra_offset=32 * hx * RH1 + j))
                        tiles.append(r)
                    rhs2_t[g] = tiles

                def load3(g):
                    h2 = h2_t.pop(g)
                    tiles = []
                    for hx in range(2):
                        r = cp.tile([128, T + 2], f8e4, tag=f"rhs3{hx}",
                                    name=f"rhs3_{g}_{hx}", bufs=3)
                        for j in range(2):
                            nc.sync.dma_start(
                                out=r[64 * j:64 * (j + 1), :],
                                in_=_ap_with(h2[:], [[RH2, 64], [1, T + 2]],
                                             extra_offset=64 * hx * RH2 + j))
                        tiles.append(r)
                    rhs3_t[g] = tiles

                # one pipeline iteration: per-tt interleave of conv2(gC2),
                # conv3(gC3), conv1(gC1) so each psum pool sees ~2.1us per
                # rotation (>> eviction latency) and the in-order PE never
                # waits on an eviction.
                def iteration(gC1, gC2, gC3, pre_fill=0, tt_fill=0):
                    h1n = h2n = None
                    if gC1 is not None:
                        h1n = cp.tile([64, RH1], f8e4, tag="h1d", name=f"h1d_{gC1}")
                        nc.vector.memset(h1n[:, 0:2], 0.0)
                        nc.vector.memset(h1n[:, T + 2:RH1], 0.0)
                        r1a, r1b = rhs1_t.pop(gC1)
                        k1, _ = gkb(gC1)
                    if gC2 is not None:
                        h2n = cp.tile([128, RH2], f8e4, tag="h2d", name=f"h2d_{gC2}")
                        nc.vector.memset(h2n[:, 0:1], 0.0)
                        nc.vector.memset(h2n[:, T + 1:RH2], 0.0)
                        r2a, r2b = rhs2_t.pop(gC2)
                        k2, _ = gkb(gC2)
                    if gC3 is not None:
                        r3a, r3b = rhs3_t.pop(gC3)
                        k3, _ = gkb(gC3)
                        h3s = cp.tile([128, 512], f8e4, tag="h3s", name=f"h3s_{gC3}")
                        acc = cp.tile([128, NT], mybir.dt.float32, tag="acc8",
                                      name=f"acc8_{gC3}")
                        acc_t[gC3] = acc
                    filler(pre_fill)
                    # tt-pair blocking: run 2 tts per stationary weight set
                    # before switching (w1c x2, w2a x2, w2b x2, w3a x2,
                    # w3b x2) -> ~2.5 weight switches/tt instead of ~5.
                    # pc2p/pc3p bufs=2 hold exactly one pair.
                    for tp in range(NT // 2):
                        pair = (2 * tp, 2 * tp + 1)
                        if gC1 is not None:
                            for tt in pair:
                                t0 = 512 * tt
                                p1 = pc1p.tile([64, 512], mybir.dt.float32,
                                               tag="pc1", name=f"pc1_{gC1}_{tt}")
                                nc.tensor.matmul(p1[0:32, :],
                                                 w1c[:, 32 * k1:32 * (k1 + 1)],
                                                 r1a[:, t0:t0 + 512],
                                                 start=True, stop=True)
                                nc.tensor.matmul(p1[32:64, :],
                                                 w1c[:, 32 * k1:32 * (k1 + 1)],
                                                 r1b[:, t0:t0 + 512],
                                                 start=True, stop=True)
                                dst = h1n[:, 2 + t0:2 + t0 + 512]
                                nc.vector.tensor_scalar(
                                    dst, p1[:], sb1d[:, k1:k1 + 1], 0.0,
                                    op0=mybir.AluOpType.add,
                                    op1=mybir.AluOpType.max)
                        if gC2 is not None:
                            p2t = {}
                            for tt in pair:
                                t0 = 512 * tt
                                p2 = pc2p.tile([128, 512], mybir.dt.float32,
                                               tag="pc2", name=f"pc2_{gC2}_{tt}")
                                p2t[tt] = p2
                                nc.tensor.matmul(p2[0:64, :],
                                                 w2a[:, 64 * k2:64 * (k2 + 1)],
                                                 r2a[:, t0:t0 + 512],
                                                 start=True, stop=False)
                                nc.tensor.matmul(p2[64:128, :],
                                                 w2a[:, 64 * k2:64 * (k2 + 1)],
                                                 r2b[:, t0:t0 + 512],
                                                 start=True, stop=False)
                            for tt in pair:
                                t0 = 512 * tt
                                p2 = p2t.pop(tt)
                                nc.tensor.matmul(p2[0:64, :],
                                                 w2b[:, 64 * k2:64 * (k2 + 1)],
                                                 r2a[0:32, t0 + 4:t0 + 4 + 512],
                                                 start=False, stop=True)
                                nc.tensor.matmul(p2[64:128, :],
                                                 w2b[:, 64 * k2:64 * (k2 + 1)],
                                                 r2b[0:32, t0 + 4:t0 + 4 + 512],
                                                 start=False, stop=True)
                                dst = h2n[:, 1 + t0:1 + t0 + 512]
                                if tt % 2 == 1:
                                    nc.scalar.activation(
                                        dst, p2[:],
                                        mybir.ActivationFunctionType.Relu,
                                        bias=sb2d[:, k2:k2 + 1])
                                else:
                                    nc.vector.tensor_scalar(
                                        dst, p2[:], sb2d[:, k2:k2 + 1], 0.0,
                                        op0=mybir.AluOpType.add,
                                        op1=mybir.AluOpType.max)
                        if gC3 is not None:
                            p3t = {}
                            for tt in pair:
                                t0 = 512 * tt
                                p3 = pc3p.tile([128, 512], mybir.dt.float32,
                                               tag="pc3", name=f"pc3_{gC3}_{tt}")
                                p3t[tt] = p3
                                nc.tensor.matmul(p3[0:64, :],
                                                 w3a[:, 64 * k3:64 * (k3 + 1)],
                                                 r3a[:, t0:t0 + 512],
                                                 start=True, stop=False)
                                nc.tensor.matmul(p3[64:128, :],
                                                 w3a[:, 64 * k3:64 * (k3 + 1)],
                                                 r3b[:, t0:t0 + 512],
                                                 start=True, stop=False)
                            for tt in pair:
                                t0 = 512 * tt
                                p3 = p3t.pop(tt)
                                nc.tensor.matmul(p3[0:64, :],
                                                 w3b[:, 64 * k3:64 * (k3 + 1)],
                                                 r3a[0:64, t0 + 2:t0 + 2 + 512],
                                                 start=False, stop=True)
                                nc.tensor.matmul(p3[64:128, :],
                                                 w3b[:, 64 * k3:64 * (k3 + 1)],
                                                 r3b[0:64, t0 + 2:t0 + 2 + 512],
                                                 start=False, stop=True)
                                nc.scalar.activation(h3s[:, 0:512], p3[:],
                                                     mybir.ActivationFunctionType.Relu,
                                                     bias=sb3d[:, k3:k3 + 1],
                                                     accum_out=acc[:, tt:tt + 1])
                        filler(2 * tt_fill)
                    if gC1 is not None and gC1 + 3 < NG:
                        load1(gC1 + 3)
                    if gC1 is not None:
                        h1_t[gC1] = h1n
                        load2(gC1)
                    if gC2 is not None:
                        h2_t[gC2] = h2n
                        load3(gC2)
                    if gC3 is not None:
                        acc = acc_t.pop(gC3)
                        nc.vector.reduce_sum(feat2[:, gC3:gC3 + 1], acc[:],
                                             axis=mybir.AxisListType.X)
                        if gC3 % 4 == 3:
                            kk = gC3 // 4
                            for hx in range(2):
                                nc.vector.tensor_copy(
                                    _ap_with(featk[kk][:], [[BL, 64], [2, 4]],
                                             extra_offset=hx),
                                    feat2[64 * hx:64 * hx + 64,
                                          4 * kk:4 * (kk + 1)])

                load1(0, nc.sync)
                load1(1, nc.sync)
                load1(2, nc.sync)
                filler(52)
                iteration(0, None, None, pre_fill=0, tt_fill=3)
                iteration(1, None, None, pre_fill=0, tt_fill=3)
                iteration(2, None, None, pre_fill=8, tt_fill=2)
                iteration(3, 0, None, pre_fill=8, tt_fill=1)
                iteration(4, 1, None, pre_fill=0, tt_fill=1)
                for i in range(5, NG):
                    iteration(i, i - 3, i - 5)
                iteration(None, NG - 3, NG - 5)
                iteration(None, NG - 2, NG - 4)
                iteration(None, NG - 1, NG - 3)
                iteration(None, None, NG - 2)
                iteration(None, None, NG - 1)

            # ---- MLP ----
            with (
                tc.tile_pool(name="mlpp", bufs=1) as mp,
                tc.tile_pool(name="mlpps", bufs=2, space=bass.MemorySpace.PSUM) as mps2,
            ):
                psh = mps2.tile([128, BL], mybir.dt.float32, tag="psh", name="psh")
                for k in range(K):
                    nc.tensor.matmul(psh[:], wc1sb[:, 128 * k:128 * (k + 1)],
                                     featk[k][:], start=(k == 0), stop=(k == K - 1))
                hmlp = mp.tile([128, BL], mybir.dt.float32, tag="hmlp", name="hmlp")
                nc.scalar.activation(hmlp[:], psh[:],
                                     mybir.ActivationFunctionType.Relu,
                                     bias=bc1sb[:, 0:1])
                pso = mps2.tile([10, BL], mybir.dt.float32, tag="pso", name="pso")
                nc.tensor.matmul(pso[:], wc2sb[:], hmlp[:], start=True, stop=True)
                osb = mp.tile([10, BL], mybir.dt.float32, tag="osb", name="osb")
                nc.scalar.activation(osb[:], pso[:],
                                     mybir.ActivationFunctionType.Identity,
                                     bias=bc2sb[:, 0:1])
                nc.sync.dma_start(out[:], osb[:])

    nc.compile()
    _NC_CACHE[nd] = nc
    return nc


def _pick_nd(h_all):
    """Smallest band half-width (in 128-blocks) covering the filter tails."""
    for nd in range(1, 16):
        cov = 128 * nd + 127
        if 2 * cov + 1 >= T:
            return nd
        tail = 0.0
        for h in h_all:
            m = np.abs(h).max()
            tail = max(tail, np.abs(h[cov + 1:T - cov]).max() / m)
        if tail < 2e-4:
            return nd
    return 15


def prepare_inputs(inputs):
    """Host folding: (nd, shared input dict, per-core xh list)."""
    x = np.asarray(inputs["x"], np.float32)
    alpha = np.asarray(inputs["alpha"], np.float32)
    tau = np.asarray(inputs["tau"], np.float32)
    omega = np.asarray(inputs["omega"], np.float32)
    W1 = np.asarray(inputs["W1"], np.float32); b1 = np.asarray(inputs["b1"], np.float32)
    g1 = np.asarray(inputs["g1"], np.float32); be1 = np.asarray(inputs["be1"], np.float32)
    W2 = np.asarray(inputs["W2"], np.float32); b2 = np.asarray(inputs["b2"], np.float32)
    g2 = np.asarray(inputs["g2"], np.float32); be2 = np.asarray(inputs["be2"], np.float32)
    W3 = np.asarray(inputs["W3"], np.float32); b3 = np.asarray(inputs["b3"], np.float32)
    g3 = np.asarray(inputs["g3"], np.float32); be3 = np.asarray(inputs["be3"], np.float32)
    Wc1 = np.asarray(inputs["Wc1"], np.float32); bc1 = np.asarray(inputs["bc1"], np.float32)
    Wc2 = np.asarray(inputs["Wc2"], np.float32); bc2 = np.asarray(inputs["bc2"], np.float32)

    H = _compute_H(alpha, tau, omega)                 # (K, F) float64
    h_all = [np.fft.irfft(H[k], n=T) for k in range(K)]
    nd = _pick_nd(h_all)
    NDB = 2 * nd + 1

    # HBall[s, d*512 + 128k + a] = h_k[(-128*(d-nd) + a - s) mod T]
    a_i = np.arange(128)[None, :]
    b_i = np.arange(128)[:, None]
    HBm = np.zeros((128, NDB * 512), np.float32)
    cov = 128 * nd + 127
    for k in range(K):
        hb = h_all[k].copy()
        if 2 * cov + 1 < T:
            hb[cov + 1:T - cov] = 0.0
        for di, d in enumerate(range(-nd, nd + 1)):
            HBm[:, di * 512 + 128 * k:di * 512 + 128 * (k + 1)] = \
                hb[(-128 * d + a_i - b_i) % T]
    HBm = HBm.astype(BF16)

    s = np.float32(1.0 / np.sqrt(1.0 + BN_EPS))
    s1 = g1 * s; s2 = g2 * s; s3 = g3 * s
    bias1 = b1 * s1 + be1                             # (K, 32)
    bias2 = b2 * s2 + be2                             # (K, 64)
    bias3 = b3 * s3 + be3                             # (K, 64)

    W1f = W1 * s1[:, :, None, None]                   # (K, o1, c, dt)
    W2f = W2 * s2[:, :, None, None]                   # (K, o2, o1, dt)
    W3f = W3 * s3[:, :, None, None]                   # (K, o3, o2, dt)

    # W1call[(7c+dt), 32k+o1]
    W1cm = np.zeros((84, K * 32), np.float32)
    for k in range(K):
        W1cm[:, 32 * k:32 * (k + 1)] = \
            np.transpose(W1f[k], (1, 2, 0)).reshape(84, 32)
    # W2aall[(32j+o1), 64k+o2] = W2f[k,o2,o1,j] j<4;  W2ball[o1, 64k+o2] dt=4
    W2am = np.zeros((128, K * 64), np.float32)
    W2bm = np.zeros((32, K * 64), np.float32)
    for k in range(K):
        W2am[:, 64 * k:64 * (k + 1)] = \
            np.transpose(W2f[k, :, :, 0:4], (2, 1, 0)).reshape(128, 64)
        W2bm[:, 64 * k:64 * (k + 1)] = W2f[k, :, :, 4].T
    # W3aall[(64j+o2), 64k+o3] = W3f[k,o3,o2,j] j<2;  W3ball[o2, 64k+o3] dt=2
    W3am = np.zeros((128, K * 64), np.float32)
    W3bm = np.zeros((64, K * 64), np.float32)
    for k in range(K):
        W3am[:, 64 * k:64 * (k + 1)] = \
            np.transpose(W3f[k, :, :, 0:2], (2, 1, 0)).reshape(128, 64)
        W3bm[:, 64 * k:64 * (k + 1)] = W3f[k, :, :, 2].T

    SB1m = np.tile(bias1.T, (2, 1)).astype(np.float32)    # (64, K)
    SB2m = np.tile(bias2.T, (2, 1)).astype(np.float32)    # (128, K)
    SB3m = np.tile(bias3.T, (2, 1)).astype(np.float32)    # (128, K)

    # Wc1m[o3, 128k+h] = Wc1[h, 64k+o3] / T   (pool-mean fold)
    Wc1m = np.zeros((64, 512), np.float32)
    for k in range(K):
        Wc1m[:, 128 * k:128 * (k + 1)] = Wc1[:, 64 * k:64 * (k + 1)].T / T
    bc1m = bc1.reshape(128, 1).astype(np.float32)
    Wc2m = np.ascontiguousarray(Wc2.T).astype(np.float32)
    bc2m = bc2.reshape(10, 1).astype(np.float32)

    shared = dict(HBall=HBm, W1call=W1cm.astype(BF16),
                  W2aall=W2am.astype(BF16), W2ball=W2bm.astype(BF16),
                  W3aall=W3am.astype(BF16), W3ball=W3bm.astype(BF16),
                  SB1d=SB1m, SB2d=SB2m, SB3d=SB3m,
                  Wc1m=Wc1m, bc1m=bc1m, Wc2m=Wc2m, bc2m=bc2m)

    # xh[p, 96j + 12b + c] = x[bglob, 128j+p, c]  (b-major cols)
    xts = []
    for core in range(NCORES):
        xl = x[BL * core:BL * (core + 1)]             # (BL, T, C)
        # (T, BL, C): row t, col (b, c) b-major = 12b + c
        xt = xl.transpose(1, 0, 2).reshape(T, BC)
        xhm = xt.reshape(32, 128, BC).transpose(1, 0, 2).reshape(128, 32 * BC)
        xts.append(np.ascontiguousarray(xhm).astype(BF16))
    return nd, shared, xts


def kernel(**inputs):
    from concourse.bass_utils import run_bass_kernel_spmd
    nd, shared, xts = prepare_inputs(inputs)
    nc = build_nc(nd)
    in_maps = [dict(shared, xh=xts[c]) for c in range(NCORES)]
    res = run_bass_kernel_spmd(nc, in_maps, list(range(NCORES)))
    logits = np.zeros((B, 10), np.float32)
    for c in range(NCORES):
        logits[BL * c:BL * (c + 1)] = np.asarray(res.results[c]["out"]).T
    return logits



# revision 39
# speedup vs baseline: 1.1505x; 1.1505x over previous
"""Trainium2 Bass kernel for nn_EnhancedUVMDModel.

Math: the UVMD Gauss-Seidel scan is linear in X = rfft(x) with real,
per-frequency coefficients, so the whole scan collapses to 4 real transfer
functions H_k(f) computed by a tiny O(K*F) host recurrence.  In the time
domain each mode is a banded circular convolution of x with h_k = irfft(H_k),
evaluated as a banded block-Toeplitz matmul (128x128 blocks, half-width nd).

Per core (8 of 64 batch samples, pure data parallel):
  phase M: modes for all (k,b) via block-Toeplitz matmuls, evicted to fp8.
  conv stack: 16 groups of (k, 2 samples).  PSUM col-packing puts both
  samples of a group in one bank (pair A at partition 0, pair B at 64),
  halving eviction instruction count.  rhs for conv1/2/3 are im2col tiles
  built by SBUF->SBUF DMA in fp8 (weights stay bf16: mixed-dtype matmuls;
  fp8 weights would blow the error budget, fp8 activations cost ~3e-3).
  conv2 = contract 128 + 32 (im2col4 + offset trick), conv3 = 128 + 64
  (im2col2 + offset trick).  Evictions fuse bias+ReLU (BN folded) and are
  split DVE/ACT; conv3 eviction accumulates the time-pool via ACT accum_out.

Clocking: the PE HAM gate gives 2.4 GHz only while the PE never idles
>~3.4us; there is NO re-warm on this silicon path once throttled (observed:
750us of dense matmuls never re-warmed).  So the kernel keeps the PE stream
gap-free: host-packed dense input DMAs, filler matmuls across the
phaseM->conv pipeline fill, deep software pipelining, and im2col source
rows laid out c-major so DMA reads spread across SBUF AXI ports
(port = fixed group of 8 partitions; b-major concentrated 12 rows on ~3
ports and starved the pipeline).

All matmul operands sit at SBUF base partition 0 (tile_position row packing
wedges the device on this HW path; col packing `out[64:128]` is fine and
HW-validated).
"""
import numpy as np
import ml_dtypes

import concourse.bass as bass
import concourse.mybir as mybir
import concourse.tile as tile
from concourse import bacc

BF16 = ml_dtypes.bfloat16
E4M3 = ml_dtypes.float8_e4m3

NCORES = 8
B, T, C, K, L = 64, 4096, 12, 4, 8
BL = B // NCORES          # 8 samples per core
BC = BL * C               # 96 rows, b-major: row = 12b + c
F = T // 2 + 1            # 2049 rfft bins
NT = T // 512             # 8 time tiles
NBLK = T // 128           # 32 conv blocks
R6 = T + 6                # modesall row section (3+3 halo)
RH1 = T + 8               # h1dual cols (2 left, 6 right halo)
RH2 = T + 4               # h2dual cols (1 left, 3 right halo)
NG = 16                   # conv groups: (k, bpair)
BN_EPS = 1e-5

_NC_CACHE = {}


def _ap_with(base, dims, extra_offset=0):
    return bass.AP(base.tensor, base.offset + extra_offset, dims,
                   base.const_val, base.runtime_checks, base.dep_tracking_offset)


def _compute_H(alpha, tau, omega):
    """Real transfer functions H_k(f): u_k_final = H_k * X.  float64."""
    freqs = np.linspace(0.0, 0.5, F)
    a = np.zeros((K, F))
    bl = np.zeros(F)
    total = np.zeros(F)
    alpha = np.asarray(alpha, np.float64)
    tau = np.asarray(tau, np.float64)
    omega = np.asarray(omega, np.float64)
    for l in range(L):
        for k in range(K):
            resid = 1.0 - (total - a[k]) + bl / 2.0
            denom = 1.0 + alpha[l, k] * (freqs - omega[k]) ** 2
            new_a = resid / denom
            total = total - a[k] + new_a
            a[k] = new_a
        bl = bl + tau[l] * (1.0 - total)
    return a                                                      # (K, F)


def build_nc(nd=1):
    if nd in _NC_CACHE:
        return _NC_CACHE[nd]
    fp32 = mybir.dt.float32
    bf16 = mybir.dt.bfloat16
    f8e4 = mybir.dt.float8e4
    nc = bacc.Bacc()
    NDB = 2 * nd + 1
    KR6 = K * R6

    xh = nc.dram_tensor("xh", [128, 32 * BC], bf16, kind="ExternalInput")
    HBall = nc.dram_tensor("HBall", [128, NDB * 512], bf16, kind="ExternalInput")
    W1call = nc.dram_tensor("W1call", [84, K * 32], bf16, kind="ExternalInput")
    W2aall = nc.dram_tensor("W2aall", [128, K * 64], bf16, kind="ExternalInput")
    W2ball = nc.dram_tensor("W2ball", [32, K * 64], bf16, kind="ExternalInput")
    W3aall = nc.dram_tensor("W3aall", [128, K * 64], bf16, kind="ExternalInput")
    W3ball = nc.dram_tensor("W3ball", [64, K * 64], bf16, kind="ExternalInput")
    SB1d = nc.dram_tensor("SB1d", [64, K], fp32, kind="ExternalInput")
    SB2d = nc.dram_tensor("SB2d", [128, K], fp32, kind="ExternalInput")
    SB3d = nc.dram_tensor("SB3d", [128, K], fp32, kind="ExternalInput")
    Wc1m = nc.dram_tensor("Wc1m", [64, 512], fp32, kind="ExternalInput")
    bc1m = nc.dram_tensor("bc1m", [128, 1], fp32, kind="ExternalInput")
    Wc2m = nc.dram_tensor("Wc2m", [128, 10], fp32, kind="ExternalInput")
    bc2m = nc.dram_tensor("bc2m", [10, 1], fp32, kind="ExternalInput")
    out = nc.dram_tensor("out", [10, BL], fp32, kind="ExternalOutput")

    with tile.TileContext(nc) as tc:
        with (
            tc.tile_pool(name="persist", bufs=1) as pp,
            tc.tile_pool(name="wpool", bufs=1) as wp,
        ):
            xsb = pp.tile([128, 32 * BC], bf16, tag="xsb", name="xsb")
            modesall = pp.tile([BC, KR6], f8e4, tag="modesall", name="modesall")
            feat2 = pp.tile([128, NG], fp32, tag="feat2", name="feat2")
            featk = [pp.tile([64, BL], fp32, tag=f"featk{k}", name=f"featk{k}")
                     for k in range(K)]

            hball = wp.tile([128, NDB * 512], bf16, tag="hball", name="hball")
            w1c = wp.tile([84, K * 32], bf16, tag="w1c", name="w1c")
            w2a = wp.tile([128, K * 64], bf16, tag="w2a", name="w2a")
            w2b = wp.tile([32, K * 64], bf16, tag="w2b", name="w2b")
            w3a = wp.tile([128, K * 64], bf16, tag="w3a", name="w3a")
            w3b = wp.tile([64, K * 64], bf16, tag="w3b", name="w3b")
            sb1d = wp.tile([64, K], fp32, tag="sb1d", name="sb1d")
            sb2d = wp.tile([128, K], fp32, tag="sb2d", name="sb2d")
            sb3d = wp.tile([128, K], fp32, tag="sb3d", name="sb3d")
            wc1sb = wp.tile([64, 512], fp32, tag="wc1", name="wc1sb")
            bc1sb = wp.tile([128, 1], fp32, tag="bc1", name="bc1sb")
            wc2sb = wp.tile([128, 10], fp32, tag="wc2", name="wc2sb")
            bc2sb = wp.tile([10, 1], fp32, tag="bc2", name="bc2sb")

            # hball + the first-needed xsb blocks (phase M starts jj=-1 ->
            # j=31, then j=0,1,..) load first so phase M starts ~8us earlier
            nc.sync.dma_start(hball[:], HBall[:])
            nc.sync.dma_start(xsb[:, 31 * BC:32 * BC], xh[:, 31 * BC:32 * BC])
            nc.sync.dma_start(xsb[:, 0:8 * BC], xh[:, 0:8 * BC])
            nc.sync.dma_start(xsb[:, 8 * BC:31 * BC], xh[:, 8 * BC:31 * BC])
            nc.sync.dma_start(w1c[:], W1call[:])
            nc.sync.dma_start(w2a[:], W2aall[:])
            nc.sync.dma_start(w2b[:], W2ball[:])
            nc.sync.dma_start(w3a[:], W3aall[:])
            nc.sync.dma_start(w3b[:], W3ball[:])
            nc.sync.dma_start(sb1d[:], SB1d[:])
            nc.sync.dma_start(sb2d[:], SB2d[:])
            nc.sync.dma_start(sb3d[:], SB3d[:])
            nc.sync.dma_start(wc1sb[:], Wc1m[:])
            nc.sync.dma_start(bc1sb[:], bc1m[:])
            nc.sync.dma_start(wc2sb[:], Wc2m[:])
            nc.sync.dma_start(bc2sb[:], bc2m[:])

            # zero mode halos (3 cols each side of each k section)
            for k in range(K):
                nc.vector.memset(modesall[:, k * R6:k * R6 + 3], 0.0)
                nc.vector.memset(modesall[:, k * R6 + T + 3:(k + 1) * R6], 0.0)

            # sync bridges: touch constant tiles on DVE/ACT once so that
            # 1-wait-slot instructions only ever wait on one semaphore later.
            scrv = wp.tile([128, 8], fp32, tag="scrv", name="scrv")
            scrs = wp.tile([128, 8], fp32, tag="scrs", name="scrs")
            nc.vector.tensor_copy(scrv[0:64, 0:1], sb1d[:, 0:1])
            nc.vector.tensor_copy(scrv[:, 1:2], sb2d[:, 0:1])
            nc.vector.tensor_copy(scrv[:, 2:3], sb3d[:, 0:1])
            nc.scalar.copy(scrs[0:64, 0:1], sb1d[:, 0:1])
            nc.scalar.copy(scrs[:, 1:2], sb2d[:, 0:1])
            nc.scalar.copy(scrs[:, 2:3], sb3d[:, 0:1])
            nc.scalar.copy(scrs[:, 3:4], bc1sb[:])
            nc.scalar.copy(scrs[0:10, 4:5], bc2sb[:])

            # ---- Phase M: modes via banded block-Toeplitz circular conv ----
            with (
                tc.tile_pool(name="mpsum", bufs=6, space=bass.MemorySpace.PSUM) as mps,
            ):
                psm = {}
                ev_i = 0
                for jj in range(-nd, NBLK + nd):
                    j = jj % NBLK
                    for d in range(-nd, nd + 1):
                        i = jj - d
                        if not (0 <= i < NBLK):
                            continue
                        if i not in psm:
                            psm[i] = mps.tile([BC, 512], mybir.dt.float32,
                                              tag="mps", name=f"mps_{i}")
                        nc.tensor.matmul(
                            psm[i][:], xsb[:, BC * j:BC * (j + 1)],
                            hball[:, 512 * (d + nd):512 * (d + nd + 1)],
                            start=(d == -nd), stop=(d == nd))
                        if d == nd:
                            pt = psm.pop(i)
                            dst = _ap_with(modesall[:],
                                           [[KR6, BC], [R6, K], [1, 128]],
                                           extra_offset=3 + 128 * i)
                            if ev_i % 2 == 0:
                                nc.vector.tensor_copy(dst, pt[:])
                            else:
                                nc.scalar.copy(dst, pt[:])
                            ev_i += 1

            # ---- Conv stack: 16 groups of (k, 2 samples) ----
            with (
                tc.tile_pool(name="warmps", bufs=1, space=bass.MemorySpace.PSUM) as wps,
                tc.tile_pool(name="pc1p", bufs=3, space=bass.MemorySpace.PSUM) as pc1p,
                tc.tile_pool(name="pc2p", bufs=2, space=bass.MemorySpace.PSUM) as pc2p,
                tc.tile_pool(name="pc3p", bufs=2, space=bass.MemorySpace.PSUM) as pc3p,
                tc.tile_pool(name="convp", bufs=2) as cp,
            ):
                wpt = wps.tile([BC, 512], mybir.dt.float32, tag="warm", name="warmps")

                def filler(n):
                    # dependency-free warm matmuls (read xsb + hball only)
                    for wi in range(n):
                        nc.tensor.matmul(wpt[:],
                                         xsb[:, BC * (wi % NBLK):BC * (wi % NBLK) + BC],
                                         hball[:, 0:512],
                                         start=True, stop=True)

                rhs1_t, h1_t, rhs2_t, h2_t, rhs3_t, acc_t = {}, {}, {}, {}, {}, {}

                def gkb(g):
                    return g // 4, g % 4              # k, bpair

                def load1(g, eng=None):
                    k, bp = gkb(g)
                    eng = eng or nc.gpsimd
                    tiles = []
                    for hx in range(2):
                        b = 2 * bp + hx
                        r = cp.tile([84, T], f8e4, tag=f"rhs1{hx}", name=f"rhs1_{g}_{hx}", bufs=3)
                        eng.dma_start(
                            out=r[:],
                            in_=_ap_with(modesall[:], [[KR6, 12], [1, 7], [1, T]],
                                         extra_offset=12 * b * KR6 + k * R6))
                        tiles.append(r)
                    rhs1_t[g] = tiles

                C2SPL = 2046   # im2col cols [0,C2SPL) need only h1 of tts 0-3

                def load2_part(g, tiles, h1, c0, clen):
                    for hx in range(2):
                        r = tiles[hx]
                        eng = nc.sync if hx == 0 else nc.scalar
                        for j in range(4):
                            eng.dma_start(
                                out=r[32 * j:32 * (j + 1), c0:c0 + clen],
                                in_=_ap_with(h1[:], [[RH1, 32], [1, clen]],
                                             extra_offset=32 * hx * RH1 + j + c0))

                def load2(g):
                    h1 = h1_t.pop(g)
                    tiles = []
                    for hx in range(2):
                        # split across the two HWDGE rings (SP + ACT) to
                        # halve the per-group im2col latency
                        eng = nc.sync if hx == 0 else nc.scalar
                        r = cp.tile([128, T + 4], f8e4, tag=f"rhs2{hx}",
                                    name=f"rhs2_{g}_{hx}")
                        for j in range(4):
                            eng.dma_start(
                                out=r[32 * j:32 * (j + 1), :],
                                in_=_ap_with(h1[:], [[RH1, 32], [1, T + 4]],
                                             extra_offset=32 * hx * RH1 + j))
                        tiles.append(r)
                    rhs2_t[g] = tiles

                def load3(g):
                    h2 = h2_t.pop(g)
                    tiles = []
                    for hx in range(2):
                        r = cp.tile([128, T + 2], f8e4, tag=f"rhs3{hx}",
                                    name=f"rhs3_{g}_{hx}")
                        for j in range(2):
                            nc.sync.dma_start(
                                out=r[64 * j:64 * (j + 1), :],
                                in_=_ap_with(h2[:], [[RH2, 64], [1, T + 2]],
                                             extra_offset=64 * hx * RH2 + j))
                        tiles.append(r)
                    rhs3_t[g] = tiles

                # one pipeline iteration: per-tt interleave of conv2(gC2),
                # conv3(gC3), conv1(gC1) so each psum pool sees ~2.1us per
                # rotation (>> eviction latency) and the in-order PE never
                # waits on an eviction.
                def iteration(gC1, gC2, gC3, pre_fill=0, tt_fill=0):
                    h1n = h2n = None
                    if gC1 is not None:
                        h1n = cp.tile([64, RH1], f8e4, tag="h1d", name=f"h1d_{gC1}")
                        nc.vector.memset(h1n[:, 0:2], 0.0)
                        nc.vector.memset(h1n[:, T + 2:RH1], 0.0)
                        r1a, r1b = rhs1_t.pop(gC1)
                        k1, _ = gkb(gC1)
                    if gC2 is not None:
                        h2n = cp.tile([128, RH2], f8e4, tag="h2d", name=f"h2d_{gC2}")
                        nc.vector.memset(h2n[:, 0:1], 0.0)
                        nc.vector.memset(h2n[:, T + 1:RH2], 0.0)
                        r2a, r2b = rhs2_t.pop(gC2)
                        k2, _ = gkb(gC2)
                    if gC3 is not None:
                        r3a, r3b = rhs3_t.pop(gC3)
                        k3, _ = gkb(gC3)
                        h3s = cp.tile([128, 512], f8e4, tag="h3s", name=f"h3s_{gC3}")
                        acc = cp.tile([128, NT], mybir.dt.float32, tag="acc8",
                                      name=f"acc8_{gC3}")
                        acc_t[gC3] = acc
                    filler(pre_fill)
                    # tt-pair blocking: run 2 tts per stationary weight set
                    # before switching (w1c x2, w2a x2, w2b x2, w3a x2,
                    # w3b x2) -> ~2.5 weight switches/tt instead of ~5.
                    # pc2p/pc3p bufs=2 hold exactly one pair.
                    for tp in range(NT // 2):
                        pair = (2 * tp, 2 * tp + 1)
                        if gC1 is not None:
                            for tt in pair:
                                t0 = 512 * tt
                                p1 = pc1p.tile([64, 512], mybir.dt.float32,
                                               tag="pc1", name=f"pc1_{gC1}_{tt}")
                                nc.tensor.matmul(p1[0:32, :],
                                                 w1c[:, 32 * k1:32 * (k1 + 1)],
                                                 r1a[:, t0:t0 + 512],
                                                 start=True, stop=True)
                                nc.tensor.matmul(p1[32:64, :],
                                                 w1c[:, 32 * k1:32 * (k1 + 1)],
                                                 r1b[:, t0:t0 + 512],
                                                 start=True, stop=True)
                                dst = h1n[:, 2 + t0:2 + t0 + 512]
                                nc.vector.tensor_scalar(
                                    dst, p1[:], sb1d[:, k1:k1 + 1], 0.0,
                                    op0=mybir.AluOpType.add,
                                    op1=mybir.AluOpType.max)
                        if gC2 is not None:
                            p2t = {}
                            for tt in pair:
                                t0 = 512 * tt
                                p2 = pc2p.tile([128, 512], mybir.dt.float32,
                                               tag="pc2", name=f"pc2_{gC2}_{tt}")
                                p2t[tt] = p2
                                nc.tensor.matmul(p2[0:64, :],
                                                 w2a[:, 64 * k2:64 * (k2 + 1)],
                                                 r2a[:, t0:t0 + 512],
                                                 start=True, stop=False)
                                nc.tensor.matmul(p2[64:128, :],
                                                 w2a[:, 64 * k2:64 * (k2 + 1)],
                                                 r2b[:, t0:t0 + 512],
                                                 start=True, stop=False)
                            for tt in pair:
                                t0 = 512 * tt
                                p2 = p2t.pop(tt)
                                nc.tensor.matmul(p2[0:64, :],
                                                 w2b[:, 64 * k2:64 * (k2 + 1)],
                                                 r2a[0:32, t0 + 4:t0 + 4 + 512],
                                                 start=False, stop=True)
                                nc.tensor.matmul(p2[64:128, :],
                                                 w2b[:, 64 * k2:64 * (k2 + 1)],
                                                 r2b[0:32, t0 + 4:t0 + 4 + 512],
                                                 start=False, stop=True)
                                dst = h2n[:, 1 + t0:1 + t0 + 512]
                                if tt % 2 == 1:
                                    nc.scalar.activation(
                                        dst, p2[:],
                                        mybir.ActivationFunctionType.Relu,
                                        bias=sb2d[:, k2:k2 + 1])
                                else:
                                    nc.vector.tensor_scalar(
                                        dst, p2[:], sb2d[:, k2:k2 + 1], 0.0,
                                        op0=mybir.AluOpType.add,
                                        op1=mybir.AluOpType.max)
                        if gC3 is not None:
                            p3t = {}
                            for tt in pair:
                                t0 = 512 * tt
                                p3 = pc3p.tile([128, 512], mybir.dt.float32,
                                               tag="pc3", name=f"pc3_{gC3}_{tt}")
                                p3t[tt] = p3
                                nc.tensor.matmul(p3[0:64, :],
                                                 w3a[:, 64 * k3:64 * (k3 + 1)],
                                                 r3a[:, t0:t0 + 512],
                                                 start=True, stop=False)
                                nc.tensor.matmul(p3[64:128, :],
                                                 w3a[:, 64 * k3:64 * (k3 + 1)],
                                                 r3b[:, t0:t0 + 512],
                                                 start=True, stop=False)
                            for tt in pair:
                                t0 = 512 * tt
                                p3 = p3t.pop(tt)
                                nc.tensor.matmul(p3[0:64, :],
                                                 w3b[:, 64 * k3:64 * (k3 + 1)],
                                                 r3a[0:64, t0 + 2:t0 + 2 + 512],
                                                 start=False, stop=True)
                                nc.tensor.matmul(p3[64:128, :],
                                                 w3b[:, 64 * k3:64 * (k3 + 1)],
                                                 r3b[0:64, t0 + 2:t0 + 2 + 512],
                                                 start=False, stop=True)
                                nc.scalar.activation(h3s[:, 0:512], p3[:],
                                                     mybir.ActivationFunctionType.Relu,
                                                     bias=sb3d[:, k3:k3 + 1],
                                                     accum_out=acc[:, tt:tt + 1])
                        if gC1 == 0 and tp == 1:
                            # first half of group-0's im2col can start now:
                            # cols [0,C2SPL) read only h1 cols written by
                            # tts 0-3 (+left halo) -> ~20us earlier issue
                            e_tiles = [cp.tile([128, T + 4], f8e4,
                                               tag=f"rhs2{hx}",
                                               name=f"rhs2_0_{hx}")
                                       for hx in range(2)]
                            load2_part(0, e_tiles, h1n, 0, C2SPL)
                        filler(2 * tt_fill)
                    if gC1 is not None and gC1 + 3 < NG:
                        load1(gC1 + 3)
                    if gC1 is not None:
                        h1_t[gC1] = h1n
                        if gC1 == 0:
                            h1_t.pop(0)
                            load2_part(0, e_tiles, h1n, C2SPL,
                                       T + 4 - C2SPL)
                            rhs2_t[0] = e_tiles
                        else:
                            load2(gC1)
                    if gC2 is not None:
                        h2_t[gC2] = h2n
                        load3(gC2)
                    if gC3 is not None:
                        acc = acc_t.pop(gC3)
                        nc.vector.reduce_sum(feat2[:, gC3:gC3 + 1], acc[:],
                                             axis=mybir.AxisListType.X)
                        if gC3 % 4 == 3:
                            kk = gC3 // 4
                            for hx in range(2):
                                nc.vector.tensor_copy(
                                    _ap_with(featk[kk][:], [[BL, 64], [2, 4]],
                                             extra_offset=hx),
                                    feat2[64 * hx:64 * hx + 64,
                                          4 * kk:4 * (kk + 1)])

                load1(0, nc.sync)
                load1(1, nc.sync)
                load1(2, nc.sync)
                filler(52)
                iteration(0, None, None, pre_fill=0, tt_fill=3)
                iteration(1, None, None, pre_fill=0, tt_fill=3)
                iteration(2, 0, None, pre_fill=8, tt_fill=1)
                iteration(3, 1, None, pre_fill=0, tt_fill=1)
                iteration(4, 2, 0, pre_fill=8)
                for i in range(5, NG):
                    iteration(i, i - 2, i - 4)
                iteration(None, NG - 2, NG - 4)
                iteration(None, NG - 1, NG - 3)
                iteration(None, None, NG - 2)
                iteration(None, None, NG - 1)

            # ---- MLP ----
            with (
                tc.tile_pool(name="mlpp", bufs=1) as mp,
                tc.tile_pool(name="mlpps", bufs=2, space=bass.MemorySpace.PSUM) as mps2,
            ):
                psh = mps2.tile([128, BL], mybir.dt.float32, tag="psh", name="psh")
                for k in range(K):
                    nc.tensor.matmul(psh[:], wc1sb[:, 128 * k:128 * (k + 1)],
                                     featk[k][:], start=(k == 0), stop=(k == K - 1))
                hmlp = mp.tile([128, BL], mybir.dt.float32, tag="hmlp", name="hmlp")
                nc.scalar.activation(hmlp[:], psh[:],
                                     mybir.ActivationFunctionType.Relu,
                                     bias=bc1sb[:, 0:1])
                pso = mps2.tile([10, BL], mybir.dt.float32, tag="pso", name="pso")
                nc.tensor.matmul(pso[:], wc2sb[:], hmlp[:], start=True, stop=True)
                osb = mp.tile([10, BL], mybir.dt.float32, tag="osb", name="osb")
                nc.scalar.activation(osb[:], pso[:],
                                     mybir.ActivationFunctionType.Identity,
                                     bias=bc2sb[:, 0:1])
                nc.sync.dma_start(out[:], osb[:])

    nc.compile()
    _NC_CACHE[nd] = nc
    return nc


def _pick_nd(h_all):
    """Smallest band half-width (in 128-blocks) covering the filter tails."""
    for nd in range(1, 16):
        cov = 128 * nd + 127
        if 2 * cov + 1 >= T:
            return nd
        tail = 0.0
        for h in h_all:
            m = np.abs(h).max()
            tail = max(tail, np.abs(h[cov + 1:T - cov]).max() / m)
        if tail < 2e-4:
            return nd
    return 15


def prepare_inputs(inputs):
    """Host folding: (nd, shared input dict, per-core xh list)."""
    x = np.asarray(inputs["x"], np.float32)
    alpha = np.asarray(inputs["alpha"], np.float32)
    tau = np.asarray(inputs["tau"], np.float32)
    omega = np.asarray(inputs["omega"], np.float32)
    W1 = np.asarray(inputs["W1"], np.float32); b1 = np.asarray(inputs["b1"], np.float32)
    g1 = np.asarray(inputs["g1"], np.float32); be1 = np.asarray(inputs["be1"], np.float32)
    W2 = np.asarray(inputs["W2"], np.float32); b2 = np.asarray(inputs["b2"], np.float32)
    g2 = np.asarray(inputs["g2"], np.float32); be2 = np.asarray(inputs["be2"], np.float32)
    W3 = np.asarray(inputs["W3"], np.float32); b3 = np.asarray(inputs["b3"], np.float32)
    g3 = np.asarray(inputs["g3"], np.float32); be3 = np.asarray(inputs["be3"], np.float32)
    Wc1 = np.asarray(inputs["Wc1"], np.float32); bc1 = np.asarray(inputs["bc1"], np.float32)
    Wc2 = np.asarray(inputs["Wc2"], np.float32); bc2 = np.asarray(inputs["bc2"], np.float32)

    H = _compute_H(alpha, tau, omega)                 # (K, F) float64
    h_all = [np.fft.irfft(H[k], n=T) for k in range(K)]
    nd = _pick_nd(h_all)
    NDB = 2 * nd + 1

    # HBall[s, d*512 + 128k + a] = h_k[(-128*(d-nd) + a - s) mod T]
    a_i = np.arange(128)[None, :]
    b_i = np.arange(128)[:, None]
    HBm = np.zeros((128, NDB * 512), np.float32)
    cov = 128 * nd + 127
    for k in range(K):
        hb = h_all[k].copy()
        if 2 * cov + 1 < T:
            hb[cov + 1:T - cov] = 0.0
        for di, d in enumerate(range(-nd, nd + 1)):
            HBm[:, di * 512 + 128 * k:di * 512 + 128 * (k + 1)] = \
                hb[(-128 * d + a_i - b_i) % T]
    HBm = HBm.astype(BF16)

    s = np.float32(1.0 / np.sqrt(1.0 + BN_EPS))
    s1 = g1 * s; s2 = g2 * s; s3 = g3 * s
    bias1 = b1 * s1 + be1                             # (K, 32)
    bias2 = b2 * s2 + be2                             # (K, 64)
    bias3 = b3 * s3 + be3                             # (K, 64)

    W1f = W1 * s1[:, :, None, None]                   # (K, o1, c, dt)
    W2f = W2 * s2[:, :, None, None]                   # (K, o2, o1, dt)
    W3f = W3 * s3[:, :, None, None]                   # (K, o3, o2, dt)

    # W1call[(7c+dt), 32k+o1]
    W1cm = np.zeros((84, K * 32), np.float32)
    for k in range(K):
        W1cm[:, 32 * k:32 * (k + 1)] = \
            np.transpose(W1f[k], (1, 2, 0)).reshape(84, 32)
    # W2aall[(32j+o1), 64k+o2] = W2f[k,o2,o1,j] j<4;  W2ball[o1, 64k+o2] dt=4
    W2am = np.zeros((128, K * 64), np.float32)
    W2bm = np.zeros((32, K * 64), np.float32)
    for k in range(K):
        W2am[:, 64 * k:64 * (k + 1)] = \
            np.transpose(W2f[k, :, :, 0:4], (2, 1, 0)).reshape(128, 64)
        W2bm[:, 64 * k:64 * (k + 1)] = W2f[k, :, :, 4].T
    # W3aall[(64j+o2), 64k+o3] = W3f[k,o3,o2,j] j<2;  W3ball[o2, 64k+o3] dt=2
    W3am = np.zeros((128, K * 64), np.float32)
    W3bm = np.zeros((64, K * 64), np.float32)
    for k in range(K):
        W3am[:, 64 * k:64 * (k + 1)] = \
            np.transpose(W3f[k, :, :, 0:2], (2, 1, 0)).reshape(128, 64)
        W3bm[:, 64 * k:64 * (k + 1)] = W3f[k, :, :, 2].T

    SB1m = np.tile(bias1.T, (2, 1)).astype(np.float32)    # (64, K)
    SB2m = np.tile(bias2.T, (2, 1)).astype(np.float32)    # (128, K)
    SB3m = np.tile(bias3.T, (2, 1)).astype(np.float32)    # (128, K)

    # Wc1m[o3, 128k+h] = Wc1[h, 64k+o3] / T   (pool-mean fold)
    Wc1m = np.zeros((64, 512), np.float32)
    for k in range(K):
        Wc1m[:, 128 * k:128 * (k + 1)] = Wc1[:, 64 * k:64 * (k + 1)].T / T
    bc1m = bc1.reshape(128, 1).astype(np.float32)
    Wc2m = np.ascontiguousarray(Wc2.T).astype(np.float32)
    bc2m = bc2.reshape(10, 1).astype(np.float32)

    shared = dict(HBall=HBm, W1call=W1cm.astype(BF16),
                  W2aall=W2am.astype(BF16), W2ball=W2bm.astype(BF16),
                  W3aall=W3am.astype(BF16), W3ball=W3bm.astype(BF16),
                  SB1d=SB1m, SB2d=SB2m, SB3d=SB3m,
                  Wc1m=Wc1m, bc1m=bc1m, Wc2m=Wc2m, bc2m=bc2m)

    # xh[p, 96j + 12b + c] = x[bglob, 128j+p, c]  (b-major cols)
    xts = []
    for core in range(NCORES):
        xl = x[BL * core:BL * (core + 1)]             # (BL, T, C)
        # (T, BL, C): row t, col (b, c) b-major = 12b + c
        xt = xl.transpose(1, 0, 2).reshape(T, BC)
        xhm = xt.reshape(32, 128, BC).transpose(1, 0, 2).reshape(128, 32 * BC)
        xts.append(np.ascontiguousarray(xhm).astype(BF16))
    return nd, shared, xts


def kernel(**inputs):
    from concourse.bass_utils import run_bass_kernel_spmd
    nd, shared, xts = prepare_inputs(inputs)
    nc = build_nc(nd)
    in_maps = [dict(shared, xh=xts[c]) for c in range(NCORES)]
    res = run_bass_kernel_spmd(nc, in_maps, list(range(NCORES)))
    logits = np.zeros((B, 10), np.float32)
    for c in range(NCORES):
        logits[BL * c:BL * (c + 1)] = np.asarray(res.results[c]["out"]).T
    return logits

